# revision 1
# baseline (speedup 1.0000x reference)
"""Trainium2 Bass kernel for ContextualAttention (two_input=False path).

Math (B=128, C=512, n_iter=128, per iteration n):
    scores[n,b,o,0] = 10 * sum_c mid[b,c,2n]   * left_cat[o,c,2n+1]
    scores[n,b,o,1] = 10 * sum_c (mid[b,c,2n]*left_cat[o,c,2n]
                                  + mid[b,c,2n+1]*left_cat[o,c,2n+1])
    att = softmax(scores, axis=o)                                # [n,B,128,2]
    out0[b,c,3n+t] = att[n,b,c,t] (c<128, else 0); out0[b,c,3n+2] = sc00[b,c,n]
    out1 same with sc10. sc01/sc11 unused.

Only the att values need device compute; the sc/zero interleave is pure host
data movement. Sharding: data-parallel over the n axis, 16 iterations per core
(core k owns n in [16k, 16k+16), i.e. l-window [32k, 32k+32) of mid/left_cat).

Device kernel per core: matmuls contract over c in 4 chunks of 128 partitions.
fp32 operands are split on the host into bf16 hi/lo pairs; each score matmul
runs as the 3-pass compensated product Mh*Lh + Mh*Ll + Ml*Lh (the dropped
Ml*Ll term is ~2^-18 relative), which streams at full bf16 rate instead of
fp32's 2x half-rate passes. Softmax: row-max (negated) via DVE feeds the exp
activation bias on ScalarE; the host divides by the per-row sum (the max
shift cancels) and assembles the full outputs.
"""

import os
from functools import lru_cache

import ml_dtypes
import numpy as np

import concourse.bacc as bacc
import concourse.mybir as mybir
import concourse.tile as tile
from concourse.bass_utils import run_bass_kernel_spmd

N_CORES = 8
B = 128          # batch rows (= out partition) and also conv out channels o
C = 512          # contraction dim
NPC = 16         # iterations n per core
LW = 2 * NPC     # l-window per core (32)
NBATCH = NPC // 2  # device batches per core; each batch covers 2 iterations
SCALE = 10.0     # softmax scale, folded into mid on the host
BF16 = ml_dtypes.bfloat16

# Results of the last run (exec_time_ns etc.), for the local test harness.
last_results = None


@lru_cache(maxsize=1)
def build_program():
    """One SPMD program; all 8 cores run it on their own shard."""
    nc = bacc.Bacc(None, target_bir_lowering=False, debug=False)
    f32 = mybir.dt.float32
    bf16 = mybir.dt.bfloat16

    # Host-prepped layouts, per core (h: 0 = bf16 hi, 1 = bf16 lo):
    #   m_t[c, l, h, b] = split(10 * mid[b, c, 32k + l])     [512, 32, 2, 128]
    #   l_t[c, l, h, o] = split(left_cat[o, c, 32k + l])     [512, 32, 2, 128]
    m_t = nc.dram_tensor("m_t", [C, LW, 2, B], bf16, kind="ExternalInput")
    l_t = nc.dram_tensor("l_t", [C, LW, 2, B], bf16, kind="ExternalInput")
    # att[b, n'*256 + t*128 + o] = exp(scores - rowmax)   (unnormalized)
    att = nc.dram_tensor("att", [B, NPC * 2 * B], f32, kind="ExternalOutput")

    # [c, cc, l, h, b] view: partition dim = c within a 128-chunk.
    m_r = m_t[:].rearrange("(cc c) l h b -> c cc l h b", cc=4)
    l_r = l_t[:].rearrange("(cc c) l h b -> c cc l h b", cc=4)

    with tile.TileContext(nc) as tc:
        with (
            # bufs=4: all four input tile-pairs stay resident, so no DMA
            # issue ever blocks on slot recycling mid-kernel. stat drops to
            # bufs=2 to stay under the 192 KiB/partition SBUF ceiling.
            tc.tile_pool(name="mbuf", bufs=4) as mbuf,
            tc.tile_pool(name="lbuf", bufs=4) as lbuf,
            tc.tile_pool(name="stat", bufs=2) as stat,
            tc.tile_pool(name="attb", bufs=3) as attb,
            tc.tile_pool(name="ps", bufs=3, space="PSUM") as ps,
        ):
            # Input DMAs move two batches at a time (4 KiB contiguous per
            # (partition, cc) — amortizes descriptor overhead). The very
            # first loads are split per-cc so the first matmul only waits
            # on one 256 KiB chunk. m on the SP HWDGE ring, l on the ACT
            # ring. Tiles are [128, 4cc, 8l, 2h, 128b] bf16 per pair.
            mtiles, ltiles = [], []
            for g in range(NBATCH // 2):
                mb = mbuf.tile([128, 4, 8, 2, B], bf16, tag="mb")
                lb = lbuf.tile([128, 4, 8, 2, B], bf16, tag="lb")
                mtiles.append(mb)
                ltiles.append(lb)
                lsl = l_r[:, :, 8 * g:8 * g + 8, :, :]
                msl = m_r[:, :, 8 * g:8 * g + 8, :, :]
                if g == 0:
                    for cc in range(4):
                        nc.scalar.dma_start(out=lb[:, cc, 0:4], in_=lsl[:, cc, 0:4])
                        nc.sync.dma_start(out=mb[:, cc, 0:4], in_=msl[:, cc, 0:4])
                    nc.scalar.dma_start(out=lb[:, :, 4:8], in_=lsl[:, :, 4:8])
                    nc.sync.dma_start(out=mb[:, :, 4:8], in_=msl[:, :, 4:8])
                else:
                    nc.scalar.dma_start(out=lb[:], in_=lsl)
                    nc.sync.dma_start(out=mb[:], in_=msl)

            for s in range(NBATCH):
                mb = mtiles[s // 2][:, :, 4 * (s % 2):4 * (s % 2) + 4]
                lb = ltiles[s // 2][:, :, 4 * (s % 2):4 * (s % 2) + 4]

                att_t = attb.tile([B, 4 * B], f32, tag="att")
                for sub in range(2):          # n' = 2s + sub
                    l0, l1 = 2 * sub, 2 * sub + 1
                    # psum cols 0:128 = t1 scores, 128:256 = t0 scores
                    pab = ps.tile([B, 2 * B], f32, tag=f"ps{sub}", name=f"pab{sub}")
                    for cc in range(4):
                        # fused moving [L(l0)|L(l1)] writes [t1|t0] at once
                        nc.tensor.matmul(
                            pab[:], mb[:, cc, l0, 0, :], lb[:, cc, l0:l0 + 2, 0, :],
                            start=(cc == 0), stop=False)
                        nc.tensor.matmul(
                            pab[:], mb[:, cc, l0, 0, :], lb[:, cc, l0:l0 + 2, 1, :],
                            start=False, stop=False)
                        nc.tensor.matmul(
                            pab[:], mb[:, cc, l0, 1, :], lb[:, cc, l0:l0 + 2, 0, :],
                            start=False, stop=False)
                        # t1 second term: M(l1) x L(l1)
                        nc.tensor.matmul(
                            pab[:, 0:B], mb[:, cc, l1, 0, :], lb[:, cc, l1, 0, :],
                            start=False, stop=False)
                        nc.tensor.matmul(
                            pab[:, 0:B], mb[:, cc, l1, 0, :], lb[:, cc, l1, 1, :],
                            start=False, stop=False)
                        nc.tensor.matmul(
                            pab[:, 0:B], mb[:, cc, l1, 1, :], lb[:, cc, l1, 0, :],
                            start=False, stop=(cc == 3))
                    for t in range(2):
                        half = pab[:, (1 - t) * B:(2 - t) * B]
                        nmx = stat.tile([B, 1], f32, tag=f"nmx{sub}{t}")
                        nc.vector.reduce_max(
                            out=nmx[:], in_=half,
                            axis=mybir.AxisListType.X, negate=True)
                        nc.scalar.activation(
                            att_t[:, (2 * sub + t) * B:(2 * sub + t + 1) * B],
                            half,
                            mybir.ActivationFunctionType.Exp,
                            bias=nmx[:, 0:1])
                nc.sync.dma_start(
                    out=att[:, s * 512:(s + 1) * 512], in_=att_t[:])

    nc.compile()
    return nc


def _split_hi_lo(x):
    """f32 [C, LW, B] -> bf16 [C, LW, 2, B] with x ~= hi + lo."""
    hi = x.astype(BF16)
    lo = (x - hi.astype(np.float32)).astype(BF16)
    return np.stack([hi, lo], axis=2)


def _shard_inputs(left, right, mid):
    """Per-core [c, l, h, b]-contiguous bf16 hi/lo shards; folds the softmax
    scale into mid."""
    in_maps = []
    for k in range(N_CORES):
        lo = 32 * k
        if lo < left.shape[2]:
            lsl = left[:, :, lo:lo + LW]
        else:
            lsl = right[:, :, lo - left.shape[2]:lo - left.shape[2] + LW]
        msl = mid[:, :, lo:lo + LW] * np.float32(SCALE)
        in_maps.append({
            "m_t": _split_hi_lo(np.ascontiguousarray(msl.transpose(1, 2, 0))),
            "l_t": _split_hi_lo(np.ascontiguousarray(lsl.transpose(1, 2, 0))),
        })
    return in_maps


def kernel(left, right, mid, sc00, sc01, sc10, sc11):
    global last_results
    left = np.asarray(left, dtype=np.float32)
    right = np.asarray(right, dtype=np.float32)
    mid = np.asarray(mid, dtype=np.float32)
    sc00 = np.asarray(sc00, dtype=np.float32)
    sc10 = np.asarray(sc10, dtype=np.float32)

    nc = build_program()
    in_maps = _shard_inputs(left, right, mid)
    trace = bool(int(os.environ.get("BASS_KERNEL_TRACE", "0")))
    last_results = run_bass_kernel_spmd(
        nc, in_maps, core_ids=list(range(N_CORES)), trace=trace,
    )

    # [k, b, n', t, o]
    att = np.stack([r["att"] for r in last_results.results])
    att = att.reshape(N_CORES, B, NPC, 2, B)
    att = att / att.sum(axis=4, keepdims=True)
    # -> [b, o(=c<128), n = k*NPC + n', t]
    attn = att.transpose(1, 4, 0, 2, 3).reshape(B, B, N_CORES * NPC, 2)

    Ls = sc00.shape[2]
    outs = []
    for sc in (sc00, sc10):
        out = np.zeros((B, C, Ls), np.float32)
        v = out.reshape(B, C, N_CORES * NPC, 3)
        v[:, :B, :, 0:2] = attn
        v[:, :, :, 2] = sc[:, :, :N_CORES * NPC]
        outs.append(out)
    return tuple(outs)



# revision 2
# speedup vs baseline: 1.2406x; 1.2406x over previous
"""Trainium2 Bass kernel for ContextualAttention (two_input=False path).

Math (B=128, C=512, n_iter=128, per iteration n):
    scores[n,b,o,0] = 10 * sum_c mid[b,c,2n]   * left_cat[o,c,2n+1]
    scores[n,b,o,1] = 10 * sum_c (mid[b,c,2n]*left_cat[o,c,2n]
                                  + mid[b,c,2n+1]*left_cat[o,c,2n+1])
    att = softmax(scores, axis=o)                                # [n,B,128,2]
    out0[b,c,3n+t] = att[n,b,c,t] (c<128, else 0); out0[b,c,3n+2] = sc00[b,c,n]
    out1 same with sc10. sc01/sc11 unused.

Only the att values need device compute; the sc/zero interleave is pure host
data movement. Sharding: data-parallel over the n axis, 16 iterations per core
(core k owns n in [16k, 16k+16), i.e. l-window [32k, 32k+32) of mid/left_cat).

The kernel is HBM-DMA-bound, so operands stream as single-pass fp16 (PE runs
fp16 at full bf16 rate; the 11-bit mantissa keeps the softmax within the 2e-2
gate - simulated end-to-end max error is ~1.5e-2 of scale). This halves input
traffic vs an fp32/hi-lo scheme and cuts matmul passes 3x. The softmax scale
is folded into mid on the host. Per iteration and 128-wide c-chunk the PE runs
two fused matmuls: M0 x [L0|L1] -> [t1-partial|t0], then M1 x L1 accumulated
onto the t1 half. Softmax: row-max (negated) via DVE feeds the exp activation
bias on ScalarE, which writes bf16; the host divides by the per-row sum (the
max shift cancels) and assembles the full outputs.
"""

import os
from functools import lru_cache

import ml_dtypes
import numpy as np

import concourse.bacc as bacc
import concourse.mybir as mybir
import concourse.tile as tile
from concourse.bass_utils import run_bass_kernel_spmd

N_CORES = 8
B = 128          # batch rows (= out partition) and also conv out channels o
C = 512          # contraction dim
NPC = 16         # iterations n per core
LW = 2 * NPC     # l-window per core (32)
SCALE = 10.0     # softmax scale, folded into mid on the host

# Results of the last run (exec_time_ns etc.), for the local test harness.
last_results = None


@lru_cache(maxsize=1)
def build_program():
    """One SPMD program; all 8 cores run it on their own shard."""
    nc = bacc.Bacc(None, target_bir_lowering=False, debug=False)
    f32 = mybir.dt.float32
    f16 = mybir.dt.float16
    bf16 = mybir.dt.bfloat16

    # Host-prepped layouts, per core:
    #   m_t[c, l, b] = fp16(10 * mid[b, c, 32k + l])     [512, 32, 128]
    #   l_t[c, l, o] = fp16(left_cat[o, c, 32k + l])     [512, 32, 128]
    m_t = nc.dram_tensor("m_t", [C, LW, B], f16, kind="ExternalInput")
    l_t = nc.dram_tensor("l_t", [C, LW, B], f16, kind="ExternalInput")
    # att[b, n'*256 + t*128 + o] = exp(scores - rowmax)   (unnormalized)
    att = nc.dram_tensor("att", [B, NPC * 2 * B], bf16, kind="ExternalOutput")

    # [c, cc, l, b] view: partition dim = c within a 128-chunk.
    m_r = m_t[:].rearrange("(cc c) l b -> c cc l b", cc=4)
    l_r = l_t[:].rearrange("(cc c) l b -> c cc l b", cc=4)

    with tile.TileContext(nc) as tc:
        with (
            # All input tile chunks stay resident (4 x 8KiB/partition each
            # tensor), so no DMA issue ever blocks on slot recycling.
            tc.tile_pool(name="mbuf", bufs=4) as mbuf,
            tc.tile_pool(name="lbuf", bufs=4) as lbuf,
            tc.tile_pool(name="stat", bufs=4) as stat,
            tc.tile_pool(name="attb", bufs=3) as attb,
            tc.tile_pool(name="ps", bufs=4, space="PSUM") as ps,
        ):
            # Input DMAs move 8 l-columns (4 iterations) at a time: 2 KiB
            # contiguous per (partition, cc) chunk. The very first loads are
            # split per-cc so the first matmul only waits on one 128 KiB
            # chunk. m on the SP HWDGE ring, l on the ACT ring.
            mtiles, ltiles = [], []
            for g in range(NPC // 4):
                mb = mbuf.tile([128, 4, 8, B], f16, tag="mb")
                lb = lbuf.tile([128, 4, 8, B], f16, tag="lb")
                mtiles.append(mb)
                ltiles.append(lb)
                msl = m_r[:, :, 8 * g:8 * g + 8, :]
                lsl = l_r[:, :, 8 * g:8 * g + 8, :]
                if g == 0:
                    for cc in range(4):
                        nc.scalar.dma_start(out=lb[:, cc, 0:4], in_=lsl[:, cc, 0:4])
                        nc.sync.dma_start(out=mb[:, cc, 0:4], in_=msl[:, cc, 0:4])
                    nc.scalar.dma_start(out=lb[:, :, 4:8], in_=lsl[:, :, 4:8])
                    nc.sync.dma_start(out=mb[:, :, 4:8], in_=msl[:, :, 4:8])
                else:
                    nc.scalar.dma_start(out=lb[:], in_=lsl)
                    nc.sync.dma_start(out=mb[:], in_=msl)

            for n in range(NPC):
                g, r = divmod(n, 4)
                l0 = 2 * r
                mb, lb = mtiles[g], ltiles[g]

                # psum cols 0:128 = t1 scores, 128:256 = t0 scores
                pab = ps.tile([B, 2 * B], f32, tag="ps", name=f"pab{n}")
                for cc in range(4):
                    # fused moving [L(l0)|L(l1)] writes [t1-part|t0] at once
                    nc.tensor.matmul(
                        pab[:], mb[:, cc, l0, :], lb[:, cc, l0:l0 + 2, :],
                        start=(cc == 0), stop=False)
                    # t1 second term: M(l1) x L(l1)
                    nc.tensor.matmul(
                        pab[:, 0:B], mb[:, cc, l0 + 1, :], lb[:, cc, l0 + 1, :],
                        start=False, stop=(cc == 3))

                att_t = attb.tile([B, 2 * B], bf16, tag="att")
                for t in range(2):
                    half = pab[:, (1 - t) * B:(2 - t) * B]
                    nmx = stat.tile([B, 1], f32, tag=f"nmx{t}")
                    nc.vector.reduce_max(
                        out=nmx[:], in_=half,
                        axis=mybir.AxisListType.X, negate=True)
                    nc.scalar.activation(
                        att_t[:, t * B:(t + 1) * B],
                        half,
                        mybir.ActivationFunctionType.Exp,
                        bias=nmx[:, 0:1])
                # Alternate output rings to balance in+out bytes per ring.
                eng = nc.sync if n % 2 == 0 else nc.scalar
                eng.dma_start(
                    out=att[:, n * 256:(n + 1) * 256], in_=att_t[:])

    nc.compile()
    return nc


def _shard_inputs(left, right, mid):
    """Per-core [c, l, b]-contiguous fp16 shards; folds the softmax scale
    into mid."""
    in_maps = []
    for k in range(N_CORES):
        lo = 32 * k
        if lo < left.shape[2]:
            lsl = left[:, :, lo:lo + LW]
        else:
            lsl = right[:, :, lo - left.shape[2]:lo - left.shape[2] + LW]
        msl = mid[:, :, lo:lo + LW] * np.float32(SCALE)
        in_maps.append({
            "m_t": np.ascontiguousarray(
                msl.transpose(1, 2, 0)).astype(np.float16),
            "l_t": np.ascontiguousarray(
                lsl.transpose(1, 2, 0)).astype(np.float16),
        })
    return in_maps


def kernel(left, right, mid, sc00, sc01, sc10, sc11):
    global last_results
    left = np.asarray(left, dtype=np.float32)
    right = np.asarray(right, dtype=np.float32)
    mid = np.asarray(mid, dtype=np.float32)
    sc00 = np.asarray(sc00, dtype=np.float32)
    sc10 = np.asarray(sc10, dtype=np.float32)

    nc = build_program()
    in_maps = _shard_inputs(left, right, mid)
    trace = bool(int(os.environ.get("BASS_KERNEL_TRACE", "0")))
    last_results = run_bass_kernel_spmd(
        nc, in_maps, core_ids=list(range(N_CORES)), trace=trace,
    )

    # [k, b, n', t, o]
    att = np.stack([np.asarray(r["att"], dtype=np.float32)
                    for r in last_results.results])
    att = att.reshape(N_CORES, B, NPC, 2, B)
    att = att / att.sum(axis=4, keepdims=True)
    # -> [b, o(=c<128), n = k*NPC + n', t]
    attn = att.transpose(1, 4, 0, 2, 3).reshape(B, B, N_CORES * NPC, 2)

    Ls = sc00.shape[2]
    outs = []
    for sc in (sc00, sc10):
        out = np.zeros((B, C, Ls), np.float32)
        v = out.reshape(B, C, N_CORES * NPC, 3)
        v[:, :B, :, 0:2] = attn
        v[:, :, :, 2] = sc[:, :, :N_CORES * NPC]
        outs.append(out)
    return tuple(outs)


# revision 4
# speedup vs baseline: 1.4750x; 1.1890x over previous
"""Trainium2 Bass kernel for ContextualAttention (two_input=False path).

Math (B=128, C=512, n_iter=128, per iteration n):
    scores[n,b,o,0] = 10 * sum_c mid[b,c,2n]   * left_cat[o,c,2n+1]
    scores[n,b,o,1] = 10 * sum_c (mid[b,c,2n]*left_cat[o,c,2n]
                                  + mid[b,c,2n+1]*left_cat[o,c,2n+1])
    att = softmax(scores, axis=o)                                # [n,B,128,2]
    out0[b,c,3n+t] = att[n,b,c,t] (c<128, else 0); out0[b,c,3n+2] = sc00[b,c,n]
    out1 same with sc10. sc01/sc11 unused.

Only the att values need device compute; the sc/zero interleave is pure host
data movement. Sharding: data-parallel over the n axis, 16 iterations per core
(core k owns n in [16k, 16k+16), i.e. l-window [32k, 32k+32) of mid/left_cat).

The kernel is HBM-DMA-bound, so operands stream as single-pass fp16 (PE runs
fp16 at full bf16 rate; the 11-bit mantissa keeps the softmax within the 2e-2
gate - simulated end-to-end max error is ~1.5e-2 of scale). This halves input
traffic vs an fp32/hi-lo scheme and cuts matmul passes 3x. The softmax scale
is folded into mid on the host. Per iteration and 128-wide c-chunk the PE runs
two fused matmuls: M0 x [L0|L1] -> [t1-partial|t0], then M1 x L1 accumulated
onto the t1 half. Softmax: row-max (negated) via DVE feeds the exp activation
bias on ScalarE, which writes bf16; the host divides by the per-row sum (the
max shift cancels) and assembles the full outputs.
"""

import os
from functools import lru_cache

import ml_dtypes
import numpy as np

import concourse.bacc as bacc
import concourse.mybir as mybir
import concourse.tile as tile
from concourse.bass_utils import run_bass_kernel_spmd

N_CORES = 8
B = 128          # batch rows (= out partition) and also conv out channels o
C = 512          # contraction dim
NPC = 16         # iterations n per core
LW = 2 * NPC     # l-window per core (32)
SCALE = 10.0     # softmax scale, folded into mid on the host

# Results of the last run (exec_time_ns etc.), for the local test harness.
last_results = None


@lru_cache(maxsize=1)
def build_program():
    """One SPMD program; all 8 cores run it on their own shard."""
    nc = bacc.Bacc(None, target_bir_lowering=False, debug=False)
    f32 = mybir.dt.float32
    f16 = mybir.dt.float16
    bf16 = mybir.dt.bfloat16

    # Host-prepped layouts, per core:
    #   m_t[c, l, b] = fp16(10 * mid[b, c, 32k + l])     [512, 32, 128]
    #   l_t[c, l, o] = fp16(left_cat[o, c, 32k + l])     [512, 32, 128]
    m_t = nc.dram_tensor("m_t", [C, LW, B], f16, kind="ExternalInput")
    l_t = nc.dram_tensor("l_t", [C, LW, B], f16, kind="ExternalInput")
    # att[b, n'*256 + t*128 + o] = exp(scores - rowmax)   (unnormalized)
    att = nc.dram_tensor("att", [B, NPC * 2 * B], bf16, kind="ExternalOutput")

    # [c, cc, l, b] view: partition dim = c within a 128-chunk.
    m_r = m_t[:].rearrange("(cc c) l b -> c cc l b", cc=4)
    l_r = l_t[:].rearrange("(cc c) l b -> c cc l b", cc=4)

    with tile.TileContext(nc) as tc:
        with (
            # All input tile chunks stay resident (4 x 8KiB/partition each
            # tensor), so no DMA issue ever blocks on slot recycling.
            tc.tile_pool(name="mbuf", bufs=4) as mbuf,
            tc.tile_pool(name="lbuf", bufs=4) as lbuf,
            tc.tile_pool(name="stat", bufs=4) as stat,
            tc.tile_pool(name="attb", bufs=1) as attb,
            tc.tile_pool(name="ps", bufs=6, space="PSUM") as ps,
        ):
            # Input DMAs move 8 l-columns (4 iterations) at a time: 2 KiB
            # contiguous per (partition, cc) chunk. The very first loads are
            # split per-cc so the first matmul only waits on one 128 KiB
            # chunk. m on the SP HWDGE ring, l on the ACT ring.
            mtiles, ltiles = [], []
            for g in range(NPC // 4):
                mb = mbuf.tile([128, 4, 8, B], f16, tag="mb")
                lb = lbuf.tile([128, 4, 8, B], f16, tag="lb")
                mtiles.append(mb)
                ltiles.append(lb)
                msl = m_r[:, :, 8 * g:8 * g + 8, :]
                lsl = l_r[:, :, 8 * g:8 * g + 8, :]
                if g == 0:
                    for cc in range(4):
                        nc.scalar.dma_start(out=lb[:, cc, 0:4], in_=lsl[:, cc, 0:4])
                        nc.sync.dma_start(out=mb[:, cc, 0:4], in_=msl[:, cc, 0:4])
                    nc.scalar.dma_start(out=lb[:, :, 4:8], in_=lsl[:, :, 4:8])
                    nc.sync.dma_start(out=mb[:, :, 4:8], in_=msl[:, :, 4:8])
                else:
                    nc.scalar.dma_start(out=lb[:], in_=lsl)
                    nc.sync.dma_start(out=mb[:], in_=msl)

            # One resident output tile; exp results accumulate here and are
            # flushed in 3 chunks from the (otherwise idle) GpSimd queue so
            # no output-DMA issue ever sits ahead of an ACTIVATE in the
            # scalar queue.
            att_t = attb.tile([B, NPC * 2 * B], bf16, tag="att")
            flush_after = {7: (0, 2048), 13: (2048, 3584), 15: (3584, 4096)}

            for n in range(NPC):
                g, r = divmod(n, 4)
                l0 = 2 * r
                mb, lb = mtiles[g], ltiles[g]

                # psum cols 0:128 = t1 scores, 128:256 = t0 scores
                pab = ps.tile([B, 2 * B], f32, tag="ps", name=f"pab{n}")
                for cc in range(4):
                    # fused moving [L(l0)|L(l1)] writes [t1-part|t0] at once
                    nc.tensor.matmul(
                        pab[:], mb[:, cc, l0, :], lb[:, cc, l0:l0 + 2, :],
                        start=(cc == 0), stop=False)
                    # t1 second term: M(l1) x L(l1)
                    nc.tensor.matmul(
                        pab[:, 0:B], mb[:, cc, l0 + 1, :], lb[:, cc, l0 + 1, :],
                        start=False, stop=(cc == 3))

                for t in range(2):
                    half = pab[:, (1 - t) * B:(2 - t) * B]
                    nmx = stat.tile([B, 1], f32, tag=f"nmx{t}")
                    nc.vector.reduce_max(
                        out=nmx[:], in_=half,
                        axis=mybir.AxisListType.X, negate=True)
                    nc.scalar.activation(
                        att_t[:, n * 256 + t * B:n * 256 + (t + 1) * B],
                        half,
                        mybir.ActivationFunctionType.Exp,
                        bias=nmx[:, 0:1])
                if n in flush_after:
                    c0, c1 = flush_after[n]
                    nc.gpsimd.dma_start(
                        out=att[:, c0:c1], in_=att_t[:, c0:c1])

    nc.compile()
    return nc


def _shard_inputs(left, right, mid):
    """Per-core [c, l, b]-contiguous fp16 shards; folds the softmax scale
    into mid."""
    in_maps = []
    for k in range(N_CORES):
        lo = 32 * k
        if lo < left.shape[2]:
            lsl = left[:, :, lo:lo + LW]
        else:
            lsl = right[:, :, lo - left.shape[2]:lo - left.shape[2] + LW]
        msl = mid[:, :, lo:lo + LW] * np.float32(SCALE)
        in_maps.append({
            "m_t": np.ascontiguousarray(
                msl.transpose(1, 2, 0)).astype(np.float16),
            "l_t": np.ascontiguousarray(
                lsl.transpose(1, 2, 0)).astype(np.float16),
        })
    return in_maps


def kernel(left, right, mid, sc00, sc01, sc10, sc11):
    global last_results
    left = np.asarray(left, dtype=np.float32)
    right = np.asarray(right, dtype=np.float32)
    mid = np.asarray(mid, dtype=np.float32)
    sc00 = np.asarray(sc00, dtype=np.float32)
    sc10 = np.asarray(sc10, dtype=np.float32)

    nc = build_program()
    in_maps = _shard_inputs(left, right, mid)
    trace = bool(int(os.environ.get("BASS_KERNEL_TRACE", "0")))
    last_results = run_bass_kernel_spmd(
        nc, in_maps, core_ids=list(range(N_CORES)), trace=trace,
    )

    # [k, b, n', t, o]
    att = np.stack([np.asarray(r["att"], dtype=np.float32)
                    for r in last_results.results])
    att = att.reshape(N_CORES, B, NPC, 2, B)
    att = att / att.sum(axis=4, keepdims=True)
    # -> [b, o(=c<128), n = k*NPC + n', t]
    attn = att.transpose(1, 4, 0, 2, 3).reshape(B, B, N_CORES * NPC, 2)

    Ls = sc00.shape[2]
    outs = []
    for sc in (sc00, sc10):
        out = np.zeros((B, C, Ls), np.float32)
        v = out.reshape(B, C, N_CORES * NPC, 3)
        v[:, :B, :, 0:2] = attn
        v[:, :, :, 2] = sc[:, :, :N_CORES * NPC]
        outs.append(out)
    return tuple(outs)


# revision 7
# speedup vs baseline: 1.4897x; 1.0100x over previous
"""Trainium2 Bass kernel for ContextualAttention (two_input=False path).

Math (B=128, C=512, n_iter=128, per iteration n):
    scores[n,b,o,0] = 10 * sum_c mid[b,c,2n]   * left_cat[o,c,2n+1]
    scores[n,b,o,1] = 10 * sum_c (mid[b,c,2n]*left_cat[o,c,2n]
                                  + mid[b,c,2n+1]*left_cat[o,c,2n+1])
    att = softmax(scores, axis=o)                                # [n,B,128,2]
    out0[b,c,3n+t] = att[n,b,c,t] (c<128, else 0); out0[b,c,3n+2] = sc00[b,c,n]
    out1 same with sc10. sc01/sc11 unused.

Only the att values need device compute; the sc/zero interleave is pure host
data movement. Sharding: data-parallel over the n axis, 16 iterations per core
(core k owns n in [16k, 16k+16), i.e. l-window [32k, 32k+32) of mid/left_cat).

The kernel is HBM-DMA-bound, so operands stream as single-pass fp16 (PE runs
fp16 at full bf16 rate; the 11-bit mantissa keeps the softmax within the 2e-2
gate - simulated end-to-end max error is ~1.5e-2 of scale). This halves input
traffic vs an fp32/hi-lo scheme and cuts matmul passes 3x. The softmax scale
is folded into mid on the host. Per iteration and 128-wide c-chunk the PE runs
two fused matmuls: M0 x [L0|L1] -> [t1-partial|t0], then M1 x L1 accumulated
onto the t1 half. Softmax: row-max (negated) via DVE feeds the exp activation
bias on ScalarE, which writes bf16; the host divides by the per-row sum (the
max shift cancels) and assembles the full outputs.
"""

import os
from functools import lru_cache

import ml_dtypes
import numpy as np

import concourse.bacc as bacc
import concourse.mybir as mybir
import concourse.tile as tile
from concourse.bass_utils import run_bass_kernel_spmd

N_CORES = 8
B = 128          # batch rows (= out partition) and also conv out channels o
C = 512          # contraction dim
NPC = 16         # iterations n per core
LW = 2 * NPC     # l-window per core (32)
SCALE = 10.0     # softmax scale, folded into mid on the host

# Results of the last run (exec_time_ns etc.), for the local test harness.
last_results = None


@lru_cache(maxsize=1)
def build_program():
    """One SPMD program; all 8 cores run it on their own shard."""
    nc = bacc.Bacc(None, target_bir_lowering=False, debug=False)
    f32 = mybir.dt.float32
    f16 = mybir.dt.float16
    bf16 = mybir.dt.bfloat16

    # Host-prepped layouts, per core:
    #   m_t[c, l, b] = fp16(10 * mid[b, c, 32k + l])     [512, 32, 128]
    #   l_t[c, l, o] = fp16(left_cat[o, c, 32k + l])     [512, 32, 128]
    m_t = nc.dram_tensor("m_t", [C, LW, B], f16, kind="ExternalInput")
    l_t = nc.dram_tensor("l_t", [C, LW, B], f16, kind="ExternalInput")
    # att[b, n'*256 + t*128 + o] = exp(scores - rowmax)   (unnormalized)
    att = nc.dram_tensor("att", [B, NPC * 2 * B], bf16, kind="ExternalOutput")

    # [c, cc, l, b] view: partition dim = c within a 128-chunk.
    m_r = m_t[:].rearrange("(cc c) l b -> c cc l b", cc=4)
    l_r = l_t[:].rearrange("(cc c) l b -> c cc l b", cc=4)

    with tile.TileContext(nc) as tc:
        with (
            # All input tile chunks stay resident (4 x 8KiB/partition each
            # tensor), so no DMA issue ever blocks on slot recycling.
            tc.tile_pool(name="mbuf", bufs=4) as mbuf,
            tc.tile_pool(name="lbuf", bufs=4) as lbuf,
            tc.tile_pool(name="stat", bufs=4) as stat,
            tc.tile_pool(name="attb", bufs=1) as attb,
            tc.tile_pool(name="ps", bufs=6, space="PSUM") as ps,
        ):
            # Input DMAs move 8 l-columns (4 iterations) at a time: 2 KiB
            # contiguous per (partition, cc) chunk. m's first chunk is split
            # per-cc so the first matmul only waits on one 128 KiB piece;
            # l uses halves to keep the scalar queue head short (every issue
            # there delays the first ACTIVATE). The last chunk is halved so
            # only 2 iterations of softmax remain when the stream ends.
            # m on the SP HWDGE ring, l on the ACT ring.
            mtiles, ltiles = [], []
            for g in range(NPC // 4):
                mb = mbuf.tile([128, 4, 8, B], f16, tag="mb")
                lb = lbuf.tile([128, 4, 8, B], f16, tag="lb")
                mtiles.append(mb)
                ltiles.append(lb)
                msl = m_r[:, :, 8 * g:8 * g + 8, :]
                lsl = l_r[:, :, 8 * g:8 * g + 8, :]
                if g == 0:
                    for cc in range(4):
                        nc.sync.dma_start(out=mb[:, cc, 0:4], in_=msl[:, cc, 0:4])
                    nc.sync.dma_start(out=mb[:, :, 4:8], in_=msl[:, :, 4:8])
                    nc.scalar.dma_start(out=lb[:, :, 0:4], in_=lsl[:, :, 0:4])
                    nc.scalar.dma_start(out=lb[:, :, 4:8], in_=lsl[:, :, 4:8])
                elif g == 3:
                    nc.sync.dma_start(out=mb[:, :, 0:4], in_=msl[:, :, 0:4])
                    nc.sync.dma_start(out=mb[:, :, 4:8], in_=msl[:, :, 4:8])
                    nc.scalar.dma_start(out=lb[:, :, 0:4], in_=lsl[:, :, 0:4])
                    nc.scalar.dma_start(out=lb[:, :, 4:8], in_=lsl[:, :, 4:8])
                else:
                    nc.sync.dma_start(out=mb[:], in_=msl)
                    nc.scalar.dma_start(out=lb[:], in_=lsl)

            # One resident output tile; exp results accumulate here and are
            # flushed in chunks from the sync queue (idle after its input
            # issues; HWDGE completion is faster than SWDGE) so no
            # output-DMA issue ever sits ahead of an ACTIVATE in the scalar
            # queue. The final chunk is small to shorten the write-receipt
            # tail.
            att_t = attb.tile([B, NPC * 2 * B], bf16, tag="att")
            nmx = stat.tile([B, 2 * NPC], f32, tag="nmx")
            flush_after = {7: (0, 2048), 13: (2048, 3584), 15: (3584, 4096)}

            for n in range(NPC):
                g, r = divmod(n, 4)
                l0 = 2 * r
                mb, lb = mtiles[g], ltiles[g]

                # psum cols 0:128 = t1 scores, 128:256 = t0 scores
                pab = ps.tile([B, 2 * B], f32, tag="ps", name=f"pab{n}")
                for cc in range(4):
                    # fused moving [L(l0)|L(l1)] writes [t1-part|t0] at once
                    nc.tensor.matmul(
                        pab[:], mb[:, cc, l0, :], lb[:, cc, l0:l0 + 2, :],
                        start=(cc == 0), stop=False)
                    # t1 second term: M(l1) x L(l1)
                    nc.tensor.matmul(
                        pab[:, 0:B], mb[:, cc, l0 + 1, :], lb[:, cc, l0 + 1, :],
                        start=False, stop=(cc == 3))

                for t in range(2):
                    half = pab[:, (1 - t) * B:(2 - t) * B]
                    nc.vector.reduce_max(
                        out=nmx[:, 2 * n + t:2 * n + t + 1], in_=half,
                        axis=mybir.AxisListType.X, negate=True)
                    nc.scalar.activation(
                        att_t[:, n * 256 + t * B:n * 256 + (t + 1) * B],
                        half,
                        mybir.ActivationFunctionType.Exp,
                        bias=nmx[:, 2 * n + t:2 * n + t + 1])
                if n in flush_after:
                    c0, c1 = flush_after[n]
                    nc.sync.dma_start(
                        out=att[:, c0:c1], in_=att_t[:, c0:c1])

    nc.compile()
    return nc


def _shard_inputs(left, right, mid):
    """Per-core [c, l, b]-contiguous fp16 shards; folds the softmax scale
    into mid."""
    in_maps = []
    for k in range(N_CORES):
        lo = 32 * k
        if lo < left.shape[2]:
            lsl = left[:, :, lo:lo + LW]
        else:
            lsl = right[:, :, lo - left.shape[2]:lo - left.shape[2] + LW]
        msl = mid[:, :, lo:lo + LW] * np.float32(SCALE)
        in_maps.append({
            "m_t": np.ascontiguousarray(
                msl.transpose(1, 2, 0)).astype(np.float16),
            "l_t": np.ascontiguousarray(
                lsl.transpose(1, 2, 0)).astype(np.float16),
        })
    return in_maps


def kernel(left, right, mid, sc00, sc01, sc10, sc11):
    global last_results
    left = np.asarray(left, dtype=np.float32)
    right = np.asarray(right, dtype=np.float32)
    mid = np.asarray(mid, dtype=np.float32)
    sc00 = np.asarray(sc00, dtype=np.float32)
    sc10 = np.asarray(sc10, dtype=np.float32)

    nc = build_program()
    in_maps = _shard_inputs(left, right, mid)
    trace = bool(int(os.environ.get("BASS_KERNEL_TRACE", "0")))
    last_results = run_bass_kernel_spmd(
        nc, in_maps, core_ids=list(range(N_CORES)), trace=trace,
    )

    # [k, b, n', t, o]
    att = np.stack([np.asarray(r["att"], dtype=np.float32)
                    for r in last_results.results])
    att = att.reshape(N_CORES, B, NPC, 2, B)
    att = att / att.sum(axis=4, keepdims=True)
    # -> [b, o(=c<128), n = k*NPC + n', t]
    attn = att.transpose(1, 4, 0, 2, 3).reshape(B, B, N_CORES * NPC, 2)

    Ls = sc00.shape[2]
    outs = []
    for sc in (sc00, sc10):
        out = np.zeros((B, C, Ls), np.float32)
        v = out.reshape(B, C, N_CORES * NPC, 3)
        v[:, :B, :, 0:2] = attn
        v[:, :, :, 2] = sc[:, :, :N_CORES * NPC]
        outs.append(out)
    return tuple(outs)


# revision 11
# speedup vs baseline: 1.5609x; 1.0478x over previous
"""Trainium2 Bass kernel for ContextualAttention (two_input=False path).

Math (B=128, C=512, n_iter=128, per iteration n):
    scores[n,b,o,0] = 10 * sum_c mid[b,c,2n]   * left_cat[o,c,2n+1]
    scores[n,b,o,1] = 10 * sum_c (mid[b,c,2n]*left_cat[o,c,2n]
                                  + mid[b,c,2n+1]*left_cat[o,c,2n+1])
    att = softmax(scores, axis=o)                                # [n,B,128,2]
    out0[b,c,3n+t] = att[n,b,c,t] (c<128, else 0); out0[b,c,3n+2] = sc00[b,c,n]
    out1 same with sc10. sc01/sc11 unused.

Only the att values need device compute; the sc/zero interleave is pure host
data movement. Sharding: data-parallel over the n axis, 16 iterations per core
(core k owns n in [16k, 16k+16), i.e. l-window [32k, 32k+32) of mid/left_cat).

The kernel is HBM-DMA-bound, so operands stream as single-pass fp16 (PE runs
fp16 at full bf16 rate; the 11-bit mantissa keeps the softmax within the 2e-2
gate - simulated end-to-end max error is ~1.5e-2 of scale). This halves input
traffic vs an fp32/hi-lo scheme and cuts matmul passes 3x. The softmax scale
is folded into mid on the host. Per iteration and 128-wide c-chunk the PE runs
two fused matmuls: M0 x [L0|L1] -> [t1-partial|t0], then M1 x L1 accumulated
onto the t1 half. Softmax: row-max (negated) via DVE feeds the exp activation
bias on ScalarE, which writes bf16; the host divides by the per-row sum (the
max shift cancels) and assembles the full outputs.
"""

import os
from functools import lru_cache

import ml_dtypes
import numpy as np

import concourse.bacc as bacc
import concourse.mybir as mybir
import concourse.tile as tile
from concourse.bass_utils import run_bass_kernel_spmd

N_CORES = 8
B = 128          # batch rows (= out partition) and also conv out channels o
C = 512          # contraction dim
NPC = 16         # iterations n per core
LW = 2 * NPC     # l-window per core (32)
SCALE = 10.0     # softmax scale, folded into mid on the host

# Results of the last run (exec_time_ns etc.), for the local test harness.
last_results = None


@lru_cache(maxsize=1)
def build_program():
    """One SPMD program; all 8 cores run it on their own shard."""
    nc = bacc.Bacc(None, target_bir_lowering=False, debug=False)
    f32 = mybir.dt.float32
    f16 = mybir.dt.float16
    bf16 = mybir.dt.bfloat16

    # Host-prepped layouts, per core:
    #   m_t[c, l, b] = fp16(10 * mid[b, c, 32k + l])     [512, 32, 128]
    #   l_t[c, l, o] = fp16(left_cat[o, c, 32k + l])     [512, 32, 128]
    m_t = nc.dram_tensor("m_t", [C, LW, B], f16, kind="ExternalInput")
    l_t = nc.dram_tensor("l_t", [C, LW, B], f16, kind="ExternalInput")
    # att[b, n'*256 + t*128 + o] = exp(scores - rowmax)   (unnormalized)
    att = nc.dram_tensor("att", [B, NPC * 2 * B], bf16, kind="ExternalOutput")

    # [c, cc, l, b] view: partition dim = c within a 128-chunk.
    m_r = m_t[:].rearrange("(cc c) l b -> c cc l b", cc=4)
    l_r = l_t[:].rearrange("(cc c) l b -> c cc l b", cc=4)

    with tile.TileContext(nc) as tc:
        with (
            # All input tile chunks stay resident (4 x 8KiB/partition each
            # tensor), so no DMA issue ever blocks on slot recycling.
            tc.tile_pool(name="mbuf", bufs=1) as mbuf,
            tc.tile_pool(name="lbuf", bufs=1) as lbuf,
            tc.tile_pool(name="stat", bufs=1) as stat,
            tc.tile_pool(name="attb", bufs=1) as attb,
            tc.tile_pool(name="ps", bufs=6, space="PSUM") as ps,
        ):
            # Input chunking: 5 DMAs per tensor covering l-columns
            # [0:2), [2:8), [8:24), [24:28), [28:32). The tiny first chunk
            # gets the first matmul going ~10.5us in; the halved tail leaves
            # only 2 iterations of softmax when the stream ends; the short
            # issue list keeps the scalar queue head clear so ACTIVATEs
            # start early and absorb the softmax during the stream.
            # m on the SP HWDGE ring, l on the ACT ring.
            bounds = [0, 2, 8, 24, 28, 32]
            mtiles, ltiles = [], []
            for c0, c1 in zip(bounds, bounds[1:]):
                mb = mbuf.tile([128, 4, c1 - c0, B], f16, tag=f"mb{c0}")
                lb = lbuf.tile([128, 4, c1 - c0, B], f16, tag=f"lb{c0}")
                mtiles.append(mb)
                ltiles.append(lb)
                nc.sync.dma_start(out=mb[:], in_=m_r[:, :, c0:c1, :])
                nc.scalar.dma_start(out=lb[:], in_=l_r[:, :, c0:c1, :])

            def tile_of(n):
                for i, c0 in enumerate(bounds[:-1]):
                    if 2 * n < bounds[i + 1]:
                        return mtiles[i], ltiles[i], 2 * n - c0
                raise AssertionError

            # One resident output tile; exp results accumulate here and are
            # flushed in chunks from the sync queue (idle after its input
            # issues; HWDGE completion is faster than SWDGE) so no
            # output-DMA issue ever sits ahead of an ACTIVATE in the scalar
            # queue. The final chunk is small to shorten the write-receipt
            # tail.
            att_t = attb.tile([B, NPC * 2 * B], bf16, tag="att")
            nmx = stat.tile([B, 2 * NPC], f32, tag="nmx")
            flush_after = {7: (0, 2048), 13: (2048, 3584), 15: (3584, 4096)}

            for n in range(NPC):
                mb, lb, l0 = tile_of(n)

                # psum cols 0:128 = t1 scores, 128:256 = t0 scores
                pab = ps.tile([B, 2 * B], f32, tag="ps", name=f"pab{n}")
                for cc in range(4):
                    # fused moving [L(l0)|L(l1)] writes [t1-part|t0] at once
                    nc.tensor.matmul(
                        pab[:], mb[:, cc, l0, :], lb[:, cc, l0:l0 + 2, :],
                        start=(cc == 0), stop=False)
                    # t1 second term: M(l1) x L(l1)
                    nc.tensor.matmul(
                        pab[:, 0:B], mb[:, cc, l0 + 1, :], lb[:, cc, l0 + 1, :],
                        start=False, stop=(cc == 3))

                for t in range(2):
                    half = pab[:, (1 - t) * B:(2 - t) * B]
                    nc.vector.reduce_max(
                        out=nmx[:, 2 * n + t:2 * n + t + 1], in_=half,
                        axis=mybir.AxisListType.X, negate=True)
                    nc.scalar.activation(
                        att_t[:, n * 256 + t * B:n * 256 + (t + 1) * B],
                        half,
                        mybir.ActivationFunctionType.Exp,
                        bias=nmx[:, 2 * n + t:2 * n + t + 1])
                if n in flush_after:
                    c0, c1 = flush_after[n]
                    nc.sync.dma_start(
                        out=att[:, c0:c1], in_=att_t[:, c0:c1])

    nc.compile()
    return nc


def _shard_inputs(left, right, mid):
    """Per-core [c, l, b]-contiguous fp16 shards; folds the softmax scale
    into mid."""
    in_maps = []
    for k in range(N_CORES):
        lo = 32 * k
        if lo < left.shape[2]:
            lsl = left[:, :, lo:lo + LW]
        else:
            lsl = right[:, :, lo - left.shape[2]:lo - left.shape[2] + LW]
        msl = mid[:, :, lo:lo + LW] * np.float32(SCALE)
        in_maps.append({
            "m_t": np.ascontiguousarray(
                msl.transpose(1, 2, 0)).astype(np.float16),
            "l_t": np.ascontiguousarray(
                lsl.transpose(1, 2, 0)).astype(np.float16),
        })
    return in_maps


def kernel(left, right, mid, sc00, sc01, sc10, sc11):
    global last_results
    left = np.asarray(left, dtype=np.float32)
    right = np.asarray(right, dtype=np.float32)
    mid = np.asarray(mid, dtype=np.float32)
    sc00 = np.asarray(sc00, dtype=np.float32)
    sc10 = np.asarray(sc10, dtype=np.float32)

    nc = build_program()
    in_maps = _shard_inputs(left, right, mid)
    trace = bool(int(os.environ.get("BASS_KERNEL_TRACE", "0")))
    last_results = run_bass_kernel_spmd(
        nc, in_maps, core_ids=list(range(N_CORES)), trace=trace,
    )

    # [k, b, n', t, o]
    att = np.stack([np.asarray(r["att"], dtype=np.float32)
                    for r in last_results.results])
    att = att.reshape(N_CORES, B, NPC, 2, B)
    att = att / att.sum(axis=4, keepdims=True)
    # -> [b, o(=c<128), n = k*NPC + n', t]
    attn = att.transpose(1, 4, 0, 2, 3).reshape(B, B, N_CORES * NPC, 2)

    Ls = sc00.shape[2]
    outs = []
    for sc in (sc00, sc10):
        out = np.zeros((B, C, Ls), np.float32)
        v = out.reshape(B, C, N_CORES * NPC, 3)
        v[:, :B, :, 0:2] = attn
        v[:, :, :, 2] = sc[:, :, :N_CORES * NPC]
        outs.append(out)
    return tuple(outs)
